# revision 14
# baseline (speedup 1.0000x reference)
"""Trainium2 Bass kernel for nn_DockingTimeModel (2-layer GINE GNN + mean-pool + MLP head).

Sharding: data-parallel over graphs. Core c owns graphs [512c, 512(c+1)) and their
(contiguous, since `batch` is sorted) node range. Edges are assigned to the core
owning their dst node. Per core, per GNN layer:
  1. dma_gather x[src] rows from host-compacted per-core tables (int16 indices),
  2. edge linear via PE matmuls (bias folded via ones-row),
  3. msg = relu(gather + lin) on DVE/ACT,
  4. dma_scatter_add msgs into an HBM accumulator by local dst (host-spaced dups),
  5. node MLP feat-major on PE (acc transposed on-PE), h stays fp32 throughout.
Two SPMD launches: A = layer 1 -> h1; host regroups h1 into per-core gather
tables; B = layer 2 + mean-pool (per-tile indicator matmuls -> partials ->
one dma_gather + reduce) + MLP head.
"""

import sys

sys.path.insert(0, "/opt/trn_rl_repo")

import math
from contextlib import ExitStack
from dataclasses import dataclass, field

import numpy as np

from concourse import bacc, bass, mybir, tile
from concourse import bass_utils
from concourse.masks import make_identity

F32 = mybir.dt.float32
I16 = mybir.dt.int16
AF = mybir.ActivationFunctionType
ALU = mybir.AluOpType

C = 8          # cores
P = 128        # partitions
ND = 64        # node feature dim
ED = 16        # edge feature dim
EMB = 128      # layer-2 output dim
USR = 12
DEAL_R = 32    # scatter-hazard dealing factor (dup dsts spaced >= group_len/DEAL_R)


# ---------------------------------------------------------------------------
# configuration (all data-derived; identical across cores => valid SPMD)
# ---------------------------------------------------------------------------
@dataclass
class CFG:
    TAB0: int
    TAB1: int
    CHUNK: int           # edges per gather/scatter chunk (mult of 128)
    chunk_tbl: list      # table id (0/1) per chunk
    ESH: int             # padded edge stream length = len(chunk_tbl)*CHUNK
    N_SH: int            # padded nodes per core (mult of NCH)
    NCH: int = 512       # node chunk
    GS: int = 512        # graphs per core
    GSP: int = 512       # padded graphs (mult of 128)
    GW: int = 8          # max graphs spanned by one 128-node tile
    PG: int = 2          # max node-tiles spanned by one graph
    n_pool_idx: int = 0  # padded pool gather stream length

    @property
    def NT(self):
        return self.N_SH // P

    @property
    def n_chunks(self):
        return len(self.chunk_tbl)


def _wrap16(idx, dtype=np.int16):
    """[L] -> [128, L/16] wrapped layout: idx i at partition i%16, col i//16,
    replicated across the 8 groups of 16 partitions."""
    L = len(idx)
    assert L % 16 == 0
    a = np.asarray(idx, dtype=dtype).reshape(L // 16, 16).T  # [16, L/16]
    return np.tile(a, (8, 1))


def _deal_perm(L, R=DEAL_R):
    """Permutation placing sorted position i at (i%R)*(L/R) + i//R."""
    assert L % R == 0
    S = L // R
    i = np.arange(L)
    pos = (i % R) * S + i // R
    out = np.empty(L, dtype=np.int64)
    out[pos] = i
    return out  # out[p] = sorted-index placed at position p


# ---------------------------------------------------------------------------
# host preprocessing
# ---------------------------------------------------------------------------
def _preprocess(x, edge_index, edge_attr, batch, G=4096, CHUNK=6400, TAB0=32768):
    src = np.asarray(edge_index[0], dtype=np.int64)
    dst = np.asarray(edge_index[1], dtype=np.int64)
    batch = np.asarray(batch, dtype=np.int64)
    GS = G // C
    gb_graph = np.arange(0, G + 1, GS)
    gb = np.searchsorted(batch, gb_graph)            # node offset per core [C+1]
    ncnt = np.diff(gb)
    NCH = 512
    N_SH = int(math.ceil(ncnt.max() / NCH) * NCH)

    owner = np.searchsorted(gb, dst, side="right") - 1

    cores = []
    m_list, g0_list, g1_list, mult0, mult1 = [], [], [], [1], [1]
    for c in range(C):
        em = np.nonzero(owner == c)[0]
        s_c, d_c = src[em], dst[em]
        uniq, inv = np.unique(s_c, return_inverse=True)
        m_list.append(len(uniq))
        g0_list.append(int((inv < TAB0).sum()))
        g1_list.append(int((inv >= TAB0).sum()))
        d_loc = d_c - gb[c]
        for gi, acc_ in ((0, mult0), (1, mult1)):
            sel = (inv < TAB0) if gi == 0 else (inv >= TAB0)
            if sel.any():
                acc_.append(int(np.bincount(d_loc[sel]).max()))
        cores.append((em, s_c, d_c, uniq, inv))

    max_m = max(m_list)
    assert max_m <= TAB0 + 32768, f"gather table overflow: {max_m}"
    TAB1 = int(math.ceil(max(max_m - TAB0, 128) / 128) * 128)
    # dst-unique chunks: each dst appears at most once per chunk (scatter-add
    # RMWs race within one instruction; across instructions Tile serializes)
    k0 = max(int(math.ceil(max(g0_list) / (CHUNK * 0.92))), max(mult0))
    k1 = (max(int(math.ceil(max(g1_list) / (CHUNK * 0.92))), max(mult1))
          if max(g1_list) > 0 else 0)

    chunk_tbl = [0] * k0 + [1] * k1
    GSP = max(P, int(math.ceil(GS / P) * P))
    cfg = CFG(TAB0=TAB0, TAB1=TAB1, CHUNK=CHUNK, chunk_tbl=chunk_tbl,
              ESH=(k0 + k1) * CHUNK, N_SH=N_SH, NCH=NCH, GS=GS, GSP=GSP)

    DUMP = N_SH  # dump row for padded edges (acc has N_SH + 128 rows)
    per_core = []
    for c in range(C):
        em, s_c, d_c, uniq, inv = cores[c]
        d_loc = d_c - gb[c]
        ea_c = np.asarray(edge_attr)[em].astype(np.float32)

        gidx = np.zeros(cfg.ESH, np.int16)
        didx = np.full(cfg.ESH, DUMP, np.int16)
        eaT = np.zeros((ED + 1, cfg.ESH), np.float32)
        for gi, nck, ck0 in ((0, k0, 0), (1, k1, k0)):
            if nck == 0:
                continue
            sel = np.nonzero((inv < TAB0) if gi == 0 else (inv >= TAB0))[0]
            base = 0 if gi == 0 else TAB0
            if len(sel):
                order = sel[np.argsort(d_loc[sel], kind="stable")]
                sd = d_loc[order]
                rank = np.arange(len(sd)) - np.searchsorted(sd, sd, side="left")
                cid = (rank + (sd * 2654435761) % nck) % nck
                for k in range(nck):
                    ke = order[cid == k]
                    nk = len(ke)
                    assert nk <= CHUNK, f"chunk overflow {nk} > {CHUNK}"
                    s0 = (ck0 + k) * CHUNK
                    gidx[s0:s0 + nk] = (inv[ke] - base).astype(np.int16)
                    didx[s0:s0 + nk] = d_loc[ke].astype(np.int16)
                    eaT[:ED, s0:s0 + nk] = ea_c[ke].T
                    eaT[ED, s0:s0 + nk] = 1.0

        n_c = ncnt[c]
        xT = np.zeros((ND, N_SH), np.float32)
        xT[:, :n_c] = np.asarray(x)[gb[c]:gb[c + 1]].T

        # pooling structures
        bl = batch[gb[c]:gb[c + 1]] - c * GS        # local graph id per node
        blp = np.full(N_SH, -1, np.int64)
        blp[:n_c] = bl
        NT = N_SH // P
        tiles = blp.reshape(NT, P)
        g_first = np.array([t[t >= 0].min() if (t >= 0).any() else 0 for t in tiles])
        relg = np.where(blp >= 0, blp - np.repeat(g_first, P), 255.0)
        GW_c = int((relg[blp >= 0]).max()) + 1 if n_c else 1
        cnt = np.bincount(bl, minlength=GS).astype(np.float32)
        # graph -> spanning tiles
        gstart = np.searchsorted(bl, np.arange(GS))
        gend = np.searchsorted(bl, np.arange(GS), side="right")
        t_lo, t_hi = gstart // P, np.maximum(gend - 1, gstart) // P
        PG_c = int((t_hi - t_lo + 1)[cnt > 0].max()) if (cnt > 0).any() else 1

        per_core.append(dict(
            gidx=gidx, didx=didx, eaT=eaT, xT=xT, uniq=uniq, n_c=n_c,
            relg=relg.astype(np.float32), g_first=g_first, cnt=cnt,
            t_lo=t_lo, t_hi=t_hi,
        ))

    cfg.GW = int(max((pc["relg"][pc["relg"] != 255.0]).max() + 1 if (pc["relg"] != 255.0).any() else 1
                     for pc in per_core))
    PG = int(max((pc["t_hi"] - pc["t_lo"] + 1)[pc["cnt"] > 0].max() if (pc["cnt"] > 0).any() else 1
                 for pc in per_core))
    cfg.PG = PG
    cfg.n_pool_idx = int(math.ceil(PG * cfg.GSP / 128) * 128)

    # pool gather index stream: position p*GSP + g -> partials row, ZPAD if none
    NT = cfg.NT
    ZPAD = NT * cfg.GW
    for pc in per_core:
        pidx = np.full(cfg.n_pool_idx, ZPAD, np.int16)
        for g in range(GS):
            if pc["cnt"][g] <= 0:
                continue
            tl, th = pc["t_lo"][g], pc["t_hi"][g]
            for p, t in enumerate(range(tl, th + 1)):
                rel = g - pc["g_first"][t]
                assert 0 <= rel < cfg.GW, (g, t, rel, cfg.GW)
                pidx[p * cfg.GSP + g] = t * cfg.GW + rel
        pc["pool_idx"] = pidx
        pc["cnt_gm"] = np.maximum(
            np.pad(pc["cnt"], (0, cfg.GSP - GS)), 1.0
        ).reshape(cfg.GSP // P, P).T.astype(np.float32)  # [128, GSP/128]

    # relids const [128, GW]
    relids = np.tile(np.arange(cfg.GW, dtype=np.float32), (P, 1))

    return cfg, gb, per_core, relids


def _gather_tables(cfg, per_core, table_src):
    """Build per-core [TAB0,64] / [TAB1,64] gather tables from row source."""
    out = []
    for pc in per_core:
        uniq = pc["uniq"]
        t0 = np.zeros((cfg.TAB0, ND), np.float32)
        t1 = np.zeros((cfg.TAB1, ND), np.float32)
        n0 = min(len(uniq), cfg.TAB0)
        t0[:n0] = table_src[uniq[:n0]]
        if len(uniq) > cfg.TAB0:
            t1[:len(uniq) - cfg.TAB0] = table_src[uniq[cfg.TAB0:]]
        out.append((t0, t1))
    return out


# ---------------------------------------------------------------------------
# program builders
# ---------------------------------------------------------------------------
def _edge_phase(ctx, tc, nc, cfg, tabs, gidx_d, didx_d, eaT_d, w_e, acc_ap):
    """Gather + edge-linear + relu + scatter-add into acc_ap [N_SH+128, 64]."""
    CH = cfg.CHUNK
    KB = CH // P            # 128-edge blocks per chunk
    gp = ctx.enter_context(tc.tile_pool(name="ep_gidx", bufs=2))
    xp = ctx.enter_context(tc.tile_pool(name="ep_xg", bufs=2))
    ep = ctx.enter_context(tc.tile_pool(name="ep_ea", bufs=2))
    dp = ctx.enter_context(tc.tile_pool(name="ep_didx", bufs=2))
    mp = ctx.enter_context(tc.tile_pool(name="ep_msg", bufs=2))
    pp = ctx.enter_context(tc.tile_pool(name="ep_lin", bufs=2, space="PSUM"))

    i16cols = CH // 16
    for ci, tb in enumerate(cfg.chunk_tbl):
        gix = gp.tile([P, i16cols], I16)
        nc.sync.dma_start(gix[:], gidx_d[:, ci * i16cols:(ci + 1) * i16cols])
        xg = xp.tile([P, KB * ND], F32)
        nc.gpsimd.dma_gather(
            out_ap=xg[:].rearrange("p (k e) -> p k e", e=ND),
            in_ap=tabs[tb][:],
            idxs_ap=gix[:],
            num_idxs=CH, num_idxs_reg=CH, elem_size=ND,
            single_packet=False,
        )
        eat = ep.tile([ED + 1, CH], F32)
        nc.sync.dma_start(eat[:], eaT_d[:, ci * CH:(ci + 1) * CH])
        dix = dp.tile([P, i16cols], I16)
        nc.sync.dma_start(dix[:], didx_d[:, ci * i16cols:(ci + 1) * i16cols])

        msg = mp.tile([P, KB * ND], F32)
        for g8 in range(0, KB, 8):
            nb = min(8, KB - g8)
            ps = pp.tile([P, 512], F32, tag="lin")
            for j in range(nb):
                b = g8 + j
                nc.tensor.matmul(
                    out=ps[:, j * ND:(j + 1) * ND],
                    lhsT=eat[:, b * P:(b + 1) * P],
                    rhs=w_e[:], start=True, stop=True,
                )
            sl = slice(g8 * ND, (g8 + nb) * ND)
            nc.vector.tensor_add(out=msg[:, sl], in0=xg[:, sl], in1=ps[:, :nb * ND])
            nc.scalar.activation(out=msg[:, sl], in_=msg[:, sl], func=AF.Relu)

        nc.gpsimd.dma_scatter_add(
            out_ap=acc_ap,
            in_ap=msg[:].rearrange("p (k e) -> p k e", e=ND),
            idxs_ap=dix[:],
            num_idxs=CH, num_idxs_reg=CH, elem_size=ND,
            single_packet=False,
        )


def _build_A(cfg):
    nc = bacc.Bacc("TRN2", target_bir_lowering=False, debug=False,
                   num_devices=C)
    d = {}
    d["tab0"] = nc.dram_tensor("tab0", [cfg.TAB0, ND], F32, kind="ExternalInput").ap()
    d["tab1"] = nc.dram_tensor("tab1", [cfg.TAB1, ND], F32, kind="ExternalInput").ap()
    d["gidx"] = nc.dram_tensor("gidx", [P, cfg.ESH // 16], I16, kind="ExternalInput").ap()
    d["didx"] = nc.dram_tensor("didx", [P, cfg.ESH // 16], I16, kind="ExternalInput").ap()
    d["eaT"] = nc.dram_tensor("eaT", [ED + 1, cfg.ESH], F32, kind="ExternalInput").ap()
    d["xT"] = nc.dram_tensor("xT", [ND, cfg.N_SH], F32, kind="ExternalInput").ap()
    d["w_e"] = nc.dram_tensor("w_e", [ED + 1, ND], F32, kind="ExternalInput").ap()
    d["w1"] = nc.dram_tensor("w1", [ND, ND], F32, kind="ExternalInput").ap()
    d["b1"] = nc.dram_tensor("b1", [ND, 1], F32, kind="ExternalInput").ap()
    d["w2"] = nc.dram_tensor("w2", [ND, ND], F32, kind="ExternalInput").ap()
    d["b2"] = nc.dram_tensor("b2", [ND, 1], F32, kind="ExternalInput").ap()
    h1T = nc.dram_tensor("h1T", [ND, cfg.N_SH], F32, kind="ExternalOutput").ap()

    with tile.TileContext(nc) as tc, ExitStack() as ctx:
        const = ctx.enter_context(tc.tile_pool(name="const", bufs=1))
        w_e = const.tile([ED + 1, ND], F32)
        nc.sync.dma_start(w_e[:], d["w_e"])
        w1 = const.tile([ND, ND], F32)
        nc.sync.dma_start(w1[:], d["w1"])
        b1 = const.tile([ND, 1], F32)
        nc.sync.dma_start(b1[:], d["b1"])
        w2 = const.tile([ND, ND], F32)
        nc.sync.dma_start(w2[:], d["w2"])
        b2 = const.tile([ND, 1], F32)
        nc.sync.dma_start(b2[:], d["b2"])
        ident = const.tile([P, P], F32)
        make_identity(nc, ident[:])
        zt = const.tile([P, 1024], F32)
        nc.vector.memset(zt[:], 0.0)

        dram = ctx.enter_context(tc.tile_pool(name="dram", bufs=1, space="DRAM"))
        acc = dram.tile([cfg.N_SH + P, ND], F32)
        _zero_dram_rows(nc, acc, cfg.N_SH + P, ND, zt)

        with ExitStack() as ectx:
            _edge_phase(ectx, tc, nc, cfg, (d["tab0"], d["tab1"]),
                        d["gidx"], d["didx"], d["eaT"], w_e, acc[:])

        with ExitStack() as nctx:
            _node_mlp(nctx, tc, nc, cfg, acc, d["xT"], ident,
                      w1, b1, w2, b2, ND, h1T, last_relu=True)

    nc.compile()
    return nc


def _zero_dram_rows(nc, t, rows, cols, zt):
    RB = 2048
    for r0 in range(0, rows, RB):
        rb = min(RB, rows - r0)
        nc.sync.dma_start(
            out=t[r0:r0 + rb, :].rearrange("(p r) e -> p (r e)", p=P),
            in_=zt[:, :rb * cols // P],
        )


def _node_mlp(ctx, tc, nc, cfg, acc, xT_d, ident, w1, b1, w2, b2, HID,
              outT_d, last_relu, out_sbuf_cb=None):
    """h = xT + accT; out = act2(w2.T @ relu(w1.T @ h + b1) + b2).
    HID = w1 output dim. If out_sbuf_cb is set it is called with
    (chunk_idx, out_tile [HID2, NCH]) instead of/in addition to DMA out."""
    NCH = cfg.NCH
    ap = ctx.enter_context(tc.tile_pool(name="np_acc", bufs=3))
    xp = ctx.enter_context(tc.tile_pool(name="np_x", bufs=2))
    hp = ctx.enter_context(tc.tile_pool(name="np_h", bufs=2))
    zp = ctx.enter_context(tc.tile_pool(name="np_z", bufs=2))
    op = ctx.enter_context(tc.tile_pool(name="np_o", bufs=2))
    tp = ctx.enter_context(tc.tile_pool(name="np_tp", bufs=2, space="PSUM"))
    mp = ctx.enter_context(tc.tile_pool(name="np_mm", bufs=1, space="PSUM"))

    HID2 = w2.shape[1]
    for t in range(cfg.N_SH // NCH):
        xT = xp.tile([ND, NCH], F32)
        nc.sync.dma_start(xT[:], xT_d[:, t * NCH:(t + 1) * NCH])
        hT = hp.tile([ND, NCH], F32)
        for j in range(NCH // P):
            a = ap.tile([P, ND], F32)
            nc.sync.dma_start(a[:], acc[t * NCH + j * P: t * NCH + (j + 1) * P, :])
            pt = tp.tile([ND, P], F32, tag="tp")
            nc.tensor.transpose(out=pt[:], in_=a[:], identity=ident[:])
            nc.vector.tensor_add(out=hT[:, j * P:(j + 1) * P],
                                 in0=pt[:], in1=xT[:, j * P:(j + 1) * P])
        z1p = mp.tile([HID, NCH], F32, tag="mm1")
        nc.tensor.matmul(out=z1p[:], lhsT=w1[:], rhs=hT[:], start=True, stop=True)
        z1 = zp.tile([HID, NCH], F32)
        nc.scalar.activation(out=z1[:], in_=z1p[:], func=AF.Relu, bias=b1[:])
        z2p = mp.tile([HID2, NCH], F32, tag="mm2")
        nc.tensor.matmul(out=z2p[:], lhsT=w2[:], rhs=z1[:], start=True, stop=True)
        o = op.tile([HID2, NCH], F32)
        nc.scalar.activation(out=o[:], in_=z2p[:],
                             func=AF.Relu if last_relu else AF.Identity, bias=b2[:])
        if outT_d is not None:
            nc.sync.dma_start(out=outT_d[:, t * NCH:(t + 1) * NCH], in_=o[:])
        if out_sbuf_cb is not None:
            out_sbuf_cb(t, o)


def _build_B(cfg):
    nc = bacc.Bacc("TRN2", target_bir_lowering=False, debug=False,
                   num_devices=C)
    d = {}
    d["tab0"] = nc.dram_tensor("tab0", [cfg.TAB0, ND], F32, kind="ExternalInput").ap()
    d["tab1"] = nc.dram_tensor("tab1", [cfg.TAB1, ND], F32, kind="ExternalInput").ap()
    d["gidx"] = nc.dram_tensor("gidx", [P, cfg.ESH // 16], I16, kind="ExternalInput").ap()
    d["didx"] = nc.dram_tensor("didx", [P, cfg.ESH // 16], I16, kind="ExternalInput").ap()
    d["eaT"] = nc.dram_tensor("eaT", [ED + 1, cfg.ESH], F32, kind="ExternalInput").ap()
    d["h1T"] = nc.dram_tensor("h1Ti", [ND, cfg.N_SH], F32, kind="ExternalInput").ap()
    d["w_e"] = nc.dram_tensor("w_e", [ED + 1, ND], F32, kind="ExternalInput").ap()
    d["w1"] = nc.dram_tensor("w1", [ND, EMB], F32, kind="ExternalInput").ap()
    d["b1"] = nc.dram_tensor("b1", [EMB, 1], F32, kind="ExternalInput").ap()
    d["w2"] = nc.dram_tensor("w2", [EMB, EMB], F32, kind="ExternalInput").ap()
    d["b2"] = nc.dram_tensor("b2", [EMB, 1], F32, kind="ExternalInput").ap()
    d["relg"] = nc.dram_tensor("relg", [P, cfg.NT], F32, kind="ExternalInput").ap()
    d["relids"] = nc.dram_tensor("relids", [P, cfg.GW], F32, kind="ExternalInput").ap()
    d["pool_idx"] = nc.dram_tensor("pool_idx", [P, cfg.n_pool_idx // 16], I16,
                                   kind="ExternalInput").ap()
    d["cnt_gm"] = nc.dram_tensor("cnt_gm", [P, cfg.GSP // P], F32,
                                 kind="ExternalInput").ap()
    d["usrT"] = nc.dram_tensor("usrT", [USR, cfg.GSP], F32, kind="ExternalInput").ap()
    for nm, shp in (("hw1a", [EMB, 128]), ("hw1b", [USR, 128]), ("hb1", [128, 1]),
                    ("hw2", [128, 64]), ("hb2", [64, 1]),
                    ("hw3", [64, 32]), ("hb3", [32, 1]),
                    ("hw4", [32, 16]), ("hb4", [16, 1]),
                    ("hw5", [16, 1]), ("hb5", [1, 1])):
        d[nm] = nc.dram_tensor(nm, shp, F32, kind="ExternalInput").ap()
    yT = nc.dram_tensor("yT", [1, cfg.GSP], F32, kind="ExternalOutput").ap()

    GW, PG, NT, GSP = cfg.GW, cfg.PG, cfg.NT, cfg.GSP
    NROW = NT * GW + P  # partials rows (+P zero rows for padding)

    with tile.TileContext(nc) as tc, ExitStack() as ctx:
        const = ctx.enter_context(tc.tile_pool(name="const", bufs=1))
        w_e = const.tile([ED + 1, ND], F32)
        nc.sync.dma_start(w_e[:], d["w_e"])
        w1 = const.tile([ND, EMB], F32)
        nc.sync.dma_start(w1[:], d["w1"])
        b1 = const.tile([EMB, 1], F32)
        nc.sync.dma_start(b1[:], d["b1"])
        w2 = const.tile([EMB, EMB], F32)
        nc.sync.dma_start(w2[:], d["w2"])
        b2 = const.tile([EMB, 1], F32)
        nc.sync.dma_start(b2[:], d["b2"])
        relg = const.tile([P, cfg.NT], F32)
        nc.sync.dma_start(relg[:], d["relg"])
        relids = const.tile([P, GW], F32)
        nc.sync.dma_start(relids[:], d["relids"])
        ident = const.tile([P, P], F32)
        make_identity(nc, ident[:])
        zt = const.tile([P, 1024], F32)
        nc.vector.memset(zt[:], 0.0)

        dram = ctx.enter_context(tc.tile_pool(name="dram", bufs=1, space="DRAM"))
        acc = dram.tile([cfg.N_SH + P, ND], F32)
        _zero_dram_rows(nc, acc, cfg.N_SH + P, ND, zt)
        parts = dram.tile([NROW, P], F32)
        # zero only the pad rows of partials (rest fully written)
        nc.sync.dma_start(
            out=parts[NT * GW:NT * GW + P, :].rearrange("(p r) e -> p (r e)", p=P),
            in_=zt[:, :P])

        with ExitStack() as ectx:
            _edge_phase(ectx, tc, nc, cfg, (d["tab0"], d["tab1"]),
                        d["gidx"], d["didx"], d["eaT"], w_e, acc[:])

        # node MLP + per-tile pooling partials
        with ExitStack() as nctx:
            pool_sb = nctx.enter_context(tc.tile_pool(name="pl_sb", bufs=2))
            pool_ps = nctx.enter_context(tc.tile_pool(name="pl_ps", bufs=1, space="PSUM"))
            pool_s = nctx.enter_context(tc.tile_pool(name="pl_s", bufs=2))
            pool_nm = nctx.enter_context(tc.tile_pool(name="pl_nm", bufs=2))

            def pool_cb(t, embT):
                # embT [EMB=128, NCH]: transpose each 128-node block, build S,
                # matmul partials, stash to DRAM
                npart = cfg.NCH // P
                for j in range(npart):
                    tl = t * npart + j
                    tps = pool_ps.tile([P, P], F32, tag="tpose")
                    nc.tensor.transpose(out=tps[:], in_=embT[:, j * P:(j + 1) * P],
                                        identity=ident[:])
                    enm = pool_nm.tile([P, P], F32)
                    nc.vector.tensor_copy(out=enm[:], in_=tps[:])
                    S = pool_s.tile([P, GW], F32)
                    nc.vector.tensor_tensor(
                        out=S[:], in0=relg[:, tl:tl + 1].to_broadcast([P, GW]),
                        in1=relids[:], op=ALU.is_equal)
                    pps = pool_ps.tile([GW, P], F32, tag="part")
                    nc.tensor.matmul(out=pps[:], lhsT=S[:],
                                     rhs=enm[:], start=True, stop=True)
                    psb = pool_sb.tile([GW, P], F32)
                    nc.vector.tensor_copy(out=psb[:], in_=pps[:])
                    nc.sync.dma_start(out=parts[tl * GW:(tl + 1) * GW, :],
                                      in_=psb[:])

            _node_mlp(nctx, tc, nc, cfg, acc, d["h1T"], ident,
                      w1, b1, w2, b2, EMB, None, last_relu=False,
                      out_sbuf_cb=pool_cb)

        # pool reduce + head
        with ExitStack() as hctx:
            hp = hctx.enter_context(tc.tile_pool(name="hd", bufs=1))
            hps = hctx.enter_context(tc.tile_pool(name="hd_ps", bufs=2, space="PSUM"))

            pix = hp.tile([P, cfg.n_pool_idx // 16], I16)
            nc.sync.dma_start(pix[:], d["pool_idx"])
            NPB = cfg.n_pool_idx // P
            gpo = hp.tile([P, NPB * P], F32)
            nc.gpsimd.dma_gather(
                out_ap=gpo[:].rearrange("p (k e) -> p k e", e=P),
                in_ap=parts[:], idxs_ap=pix[:],
                num_idxs=cfg.n_pool_idx, num_idxs_reg=cfg.n_pool_idx,
                elem_size=P, single_packet=False)
            GB = GSP // P  # graph blocks
            v = gpo[:].rearrange("p (q b e) -> p q b e", q=PG, b=GB)
            pooled = hp.tile([P, GB * P], F32)
            pv = pooled[:].rearrange("p (b e) -> p b e", b=GB)
            if PG == 1:
                nc.vector.tensor_copy(out=pv, in_=v[:, 0])
            else:
                nc.vector.tensor_add(out=pv, in0=v[:, 0], in1=v[:, 1])
                for q in range(2, PG):
                    nc.vector.tensor_add(out=pv, in0=pv, in1=v[:, q])
            cntg = hp.tile([P, GB], F32)
            nc.sync.dma_start(cntg[:], d["cnt_gm"])
            invc = hp.tile([P, GB], F32)
            nc.vector.reciprocal(invc[:], cntg[:])
            for b in range(GB):
                nc.vector.tensor_tensor(
                    out=pooled[:, b * P:(b + 1) * P],
                    in0=pooled[:, b * P:(b + 1) * P],
                    in1=invc[:, b:b + 1].to_broadcast([P, P]), op=ALU.mult)
            embT = hp.tile([P, GSP], F32)
            for b in range(GB):
                tps = hps.tile([P, P], F32, tag="hd")
                nc.tensor.transpose(out=tps[:], in_=pooled[:, b * P:(b + 1) * P],
                                    identity=ident[:])
                nc.vector.tensor_copy(out=embT[:, b * P:(b + 1) * P], in_=tps[:])
            usrT = hp.tile([USR, GSP], F32)
            nc.sync.dma_start(usrT[:], d["usrT"])

            hw = {nm: hp.tile(d[nm].shape, F32, name=f"t_{nm}")
                  for nm in ("hw1a", "hw1b", "hb1", "hw2", "hb2", "hw3", "hb3",
                             "hw4", "hb4", "hw5", "hb5")}
            for nm, t in hw.items():
                nc.sync.dma_start(t[:], d[nm])

            z1p = hps.tile([128, GSP], F32, tag="hd")
            nc.tensor.matmul(out=z1p[:], lhsT=hw["hw1a"][:], rhs=embT[:],
                             start=True, stop=False)
            nc.tensor.matmul(out=z1p[:], lhsT=hw["hw1b"][:], rhs=usrT[:],
                             start=False, stop=True)
            z1 = hp.tile([128, GSP], F32)
            nc.scalar.activation(out=z1[:], in_=z1p[:], func=AF.Relu, bias=hw["hb1"][:])
            z2p = hps.tile([64, GSP], F32, tag="hd")
            nc.tensor.matmul(out=z2p[:], lhsT=hw["hw2"][:], rhs=z1[:], start=True, stop=True)
            z2 = hp.tile([64, GSP], F32)
            nc.scalar.activation(out=z2[:], in_=z2p[:], func=AF.Relu, bias=hw["hb2"][:])
            z3p = hps.tile([32, GSP], F32, tag="hd")
            nc.tensor.matmul(out=z3p[:], lhsT=hw["hw3"][:], rhs=z2[:], start=True, stop=True)
            z3 = hp.tile([32, GSP], F32)
            nc.scalar.activation(out=z3[:], in_=z3p[:], func=AF.Relu, bias=hw["hb3"][:])
            z4p = hps.tile([16, GSP], F32, tag="hd")
            nc.tensor.matmul(out=z4p[:], lhsT=hw["hw4"][:], rhs=z3[:], start=True, stop=True)
            z4 = hp.tile([16, GSP], F32)
            nc.scalar.activation(out=z4[:], in_=z4p[:], func=AF.Relu, bias=hw["hb4"][:])
            z5p = hps.tile([1, GSP], F32, tag="hd")
            nc.tensor.matmul(out=z5p[:], lhsT=hw["hw5"][:], rhs=z4[:], start=True, stop=True)
            z5 = hp.tile([1, GSP], F32)
            nc.scalar.activation(out=z5[:], in_=z5p[:], func=AF.Identity, bias=hw["hb5"][:])
            nc.sync.dma_start(out=yT, in_=z5[:])

    nc.compile()
    return nc


# ---------------------------------------------------------------------------
# top-level kernel
# ---------------------------------------------------------------------------
def kernel(x, edge_index, edge_attr, batch, usr,
           e1_w, e1_b, n1_w1, n1_b1, n1_w2, n1_b2,
           e2_w, e2_b, n2_w1, n2_b1, n2_w2, n2_b2,
           h1_w, h1_b, h2_w, h2_b, h3_w, h3_b, h4_w, h4_b, h5_w, h5_b,
           _trace=False):
    x = np.asarray(x, np.float32)
    edge_attr = np.asarray(edge_attr, np.float32)
    usr = np.asarray(usr, np.float32)
    f32 = lambda a: np.ascontiguousarray(np.asarray(a, np.float32))

    cfg, gb, per_core, relids = _preprocess(x, edge_index, edge_attr, batch)

    # ---- launch A ----
    ncA = _build_A(cfg)
    tabs = _gather_tables(cfg, per_core, x)
    w_e1 = np.vstack([f32(e1_w), f32(e1_b)[None, :]])
    in_maps = []
    for c, pc in enumerate(per_core):
        in_maps.append(dict(
            tab0=tabs[c][0], tab1=tabs[c][1],
            gidx=_wrap16(pc["gidx"]), didx=_wrap16(pc["didx"]),
            eaT=np.ascontiguousarray(pc["eaT"]), xT=pc["xT"],
            w_e=w_e1, w1=f32(n1_w1), b1=f32(n1_b1)[:, None],
            w2=f32(n1_w2), b2=f32(n1_b2)[:, None],
        ))
    resA = bass_utils.run_bass_kernel_spmd(ncA, in_maps, core_ids=list(range(C)),
                                           trace=_trace)
    h1_full = np.zeros((x.shape[0], ND), np.float32)
    h1T_own = []
    for c, pc in enumerate(per_core):
        h1T = resA.results[c]["h1T"]
        h1T_own.append(h1T)
        h1_full[gb[c]:gb[c + 1]] = h1T[:, :pc["n_c"]].T

    # ---- launch B ----
    ncB = _build_B(cfg)
    tabsB = _gather_tables(cfg, per_core, h1_full)
    w_e2 = np.vstack([f32(e2_w), f32(e2_b)[None, :]])
    NT = cfg.NT
    in_mapsB = []
    for c, pc in enumerate(per_core):
        usrT = np.zeros((USR, cfg.GSP), np.float32)
        usrT[:, :cfg.GS] = usr[c * cfg.GS:(c + 1) * cfg.GS].T
        in_mapsB.append(dict(
            tab0=tabsB[c][0], tab1=tabsB[c][1],
            gidx=_wrap16(pc["gidx"]), didx=_wrap16(pc["didx"]),
            eaT=np.ascontiguousarray(pc["eaT"]), h1Ti=h1T_own[c],
            w_e=w_e2, w1=f32(n2_w1), b1=f32(n2_b1)[:, None],
            w2=f32(n2_w2), b2=f32(n2_b2)[:, None],
            relg=np.ascontiguousarray(pc["relg"].reshape(NT, P).T),
            relids=relids,
            pool_idx=_wrap16(pc["pool_idx"]),
            cnt_gm=pc["cnt_gm"],
            usrT=usrT,
            hw1a=f32(h1_w)[:EMB], hw1b=f32(h1_w)[EMB:], hb1=f32(h1_b)[:, None],
            hw2=f32(h2_w), hb2=f32(h2_b)[:, None],
            hw3=f32(h3_w), hb3=f32(h3_b)[:, None],
            hw4=f32(h4_w), hb4=f32(h4_b)[:, None],
            hw5=f32(h5_w), hb5=f32(h5_b)[:, None],
        ))
    resB = bass_utils.run_bass_kernel_spmd(ncB, in_mapsB, core_ids=list(range(C)),
                                           trace=_trace)
    out = np.concatenate([resB.results[c]["yT"][0, :cfg.GS] for c in range(C)])
    kernel._last = (resA, resB)
    return out[:, None].astype(np.float32)


# revision 15
# speedup vs baseline: 1.6820x; 1.6820x over previous
"""Trainium2 Bass kernel for nn_DockingTimeModel (2-layer GINE GNN + mean-pool
+ MLP head), single merged SPMD launch on 8 NeuronCores.

Sharding: data-parallel over graphs. Core c owns graphs [512c, 512(c+1)) and
their (contiguous, `batch` is sorted) node range; edges live on the core owning
their dst node. Per layer: dma_gather x[src] rows from host-compacted int16
tables -> edge linear on PE (bias folded via ones-row) -> relu(gather+lin) ->
dma_scatter_add into an HBM accumulator by local dst (dst-unique per chunk;
Tile serializes chunks so HBM read-modify-write never races) -> feat-major node
MLP on PE. Between layers, each core pre-gathers the h1 rows every peer needs
and exchanges them with one AllToAll; layer 2 gathers from the received
compact table. Mean-pool via per-tile indicator matmuls -> partials ->
one dma_gather + reduce; MLP head on-chip; output [1, 512] per core.
"""
import sys

sys.path.insert(0, "/opt/trn_rl_repo")

import math
from contextlib import ExitStack
from dataclasses import dataclass

import numpy as np

from concourse import bacc, bass, mybir, tile
from concourse import bass_utils
from concourse.masks import make_identity

F32 = mybir.dt.float32
I16 = mybir.dt.int16
AF = mybir.ActivationFunctionType
ALU = mybir.AluOpType

C = 8
P = 128
ND = 64
ED = 16
EMB = 128
USR = 12


def _wrap16(idx):
    L = len(idx)
    assert L % 16 == 0
    a = np.asarray(idx, np.int16).reshape(L // 16, 16).T
    return np.tile(a, (8, 1))


@dataclass
class Stream:
    """One layer's edge stream layout: groups of (n_chunks, chunk_size)."""
    groups: list          # [(k, CH, table_id)]
    ESH: int

    @property
    def chunks(self):
        out = []
        off = 0
        for k, CH, tb in self.groups:
            for i in range(k):
                out.append((off, CH, tb))
                off += CH
        return out


@dataclass
class CFG:
    TAB0: int
    TAB1: int
    s1: Stream
    s2: Stream
    N_SH: int
    B: int                # A2A block rows per (dst,src) pair
    NCH: int = 512
    GS: int = 512
    GSP: int = 512
    GW: int = 8
    PG: int = 2
    n_pool_idx: int = 0

    @property
    def NT(self):
        return self.N_SH // P


def _mk_stream(pos_all, dloc_all, ea_all, split, CHUNK_MAX):
    """Build per-core edge streams for one layer.

    pos_all[c]: gather-table row per edge (monotone groups: <split -> g0).
    Returns (Stream, per-core (gidx, didx, eaT) builder fn).
    """
    Cn = len(pos_all)
    # per-core per-group counts and dst multiplicities
    kg, chg = [], []
    for gi in range(2):
        ns, mults = [], [1]
        for c in range(Cn):
            sel = (pos_all[c] < split) if gi == 0 else (pos_all[c] >= split)
            ns.append(int(sel.sum()))
            if sel.any():
                mults.append(int(np.bincount(dloc_all[c][sel]).max()))
        if max(ns) == 0:
            kg.append(0)
            chg.append(0)
            continue
        k = max(int(math.ceil(max(ns) / (CHUNK_MAX * 0.95))), max(mults))
        # chunk size to fit worst core with ~6% slack for hash imbalance
        CH = int(math.ceil(max(ns) / k * 1.08 / 128) * 128)
        CH = max(CH, 256)
        kg.append(k)
        chg.append(CH)
    groups = []
    if kg[0]:
        groups.append((kg[0], chg[0], 0))
    if kg[1]:
        groups.append((kg[1], chg[1], 1))
    ESH = sum(k * ch for k, ch, _ in groups)
    return Stream(groups=groups, ESH=ESH)


def _fill_stream(st, pos, dloc, ea, split, DUMP, retry=4):
    """Place edges into the stream: per group, chunk id = (rank_within_dst +
    hash(dst)) % k -> dst-unique chunks. Returns (gidx, didx, eaT, cnts)."""
    gidx = np.zeros(st.ESH, np.int16)
    didx = np.full(st.ESH, DUMP, np.int16)
    eaT = np.zeros((ED + 1, st.ESH), np.float32)
    cnts = np.zeros(len(st.chunks), np.int32)
    cbase = 0
    off = 0
    for k, CH, tb in st.groups:
        base = 0 if tb == 0 else split
        sel = np.nonzero((pos < split) if tb == 0 else (pos >= split))[0]
        if len(sel):
            order = sel[np.argsort(dloc[sel], kind="stable")]
            sd = dloc[order]
            rank = np.arange(len(sd)) - np.searchsorted(sd, sd, side="left")
            for salt in range(retry):
                cid = (rank + (sd * (2654435761 + salt * 97)) % k) % k
                sizes = np.bincount(cid, minlength=k)
                if sizes.max() <= CH:
                    break
            else:
                raise AssertionError(f"chunk overflow {sizes.max()} > {CH}")
            for ki in range(k):
                ke = order[cid == ki]
                s0 = off + ki * CH
                nk = len(ke)
                # ucode needs >= 1 valid index per call
                if nk == 0:
                    gidx[s0] = 0
                    didx[s0] = DUMP
                    nk = 1
                else:
                    gidx[s0:s0 + nk] = (pos[ke] - base).astype(np.int16)
                    didx[s0:s0 + nk] = dloc[ke].astype(np.int16)
                    eaT[:ED, s0:s0 + nk] = ea[ke].T
                    eaT[ED, s0:s0 + nk] = 1.0
                cnts[cbase + ki] = nk
        else:
            gidx[off::CH] = 0
            didx[off::CH] = DUMP
            cnts[cbase:cbase + k] = 1
        cbase += k
        off += k * CH
    return gidx, didx, eaT, cnts


def _preprocess(x, edge_index, edge_attr, batch, G=4096, CHUNK_MAX=6400,
                TAB0=32768):
    src = np.asarray(edge_index[0], np.int64)
    dst = np.asarray(edge_index[1], np.int64)
    batch = np.asarray(batch, np.int64)
    ea = np.asarray(edge_attr, np.float32)
    GS = G // C
    gb = np.searchsorted(batch, np.arange(0, G + 1, GS))
    ncnt = np.diff(gb)
    NCH = 512
    N_SH = int(math.ceil(ncnt.max() / NCH) * NCH)
    owner = np.searchsorted(gb, dst, side="right") - 1

    cores = []
    for c in range(C):
        em = np.nonzero(owner == c)[0]
        s_c, d_c = src[em], dst[em]
        uniq, inv = np.unique(s_c, return_inverse=True)
        cores.append(dict(em=em, uniq=uniq, inv=inv, dloc=d_c - gb[c],
                          ea=ea[em]))
    max_m = max(len(pc["uniq"]) for pc in cores)
    assert max_m <= TAB0 + 32768
    TAB1 = int(math.ceil(max(max_m - TAB0, 128) / 128) * 128)

    # A2A block size: rows core c needs from owner o
    need = np.zeros((C, C), np.int64)
    for c in range(C):
        own = np.searchsorted(gb, cores[c]["uniq"], side="right") - 1
        cores[c]["uniq_owner"] = own
        for o in range(C):
            need[c, o] = int((own == o).sum())
    B = int(math.ceil((need.max() + 1) / 128) * 128)
    assert C * B <= TAB0 + 32768, f"A2A table too large: {C * B}"

    # L2 table position per uniq row: block(owner)*B + rank within block
    for c in range(C):
        own = cores[c]["uniq_owner"]
        r = np.zeros(len(own), np.int64)
        for o in range(C):
            m = own == o
            r[m] = np.arange(m.sum())
        cores[c]["pos2"] = (own * B + r)[cores[c]["inv"]]  # per-edge

    s1 = _mk_stream([pc["inv"] for pc in cores],
                    [pc["dloc"] for pc in cores],
                    None, TAB0, CHUNK_MAX)
    s2 = _mk_stream([pc["pos2"] for pc in cores],
                    [pc["dloc"] for pc in cores],
                    None, TAB0, CHUNK_MAX)

    GSP = max(P, int(math.ceil(GS / P) * P))
    cfg = CFG(TAB0=TAB0, TAB1=TAB1, s1=s1, s2=s2, N_SH=N_SH, B=B,
              NCH=NCH, GS=GS, GSP=GSP)

    DUMP = N_SH
    per_core = []
    for c in range(C):
        pc = cores[c]
        g1 = _fill_stream(s1, pc["inv"], pc["dloc"], pc["ea"], TAB0, DUMP)
        g2 = _fill_stream(s2, pc["pos2"], pc["dloc"], pc["ea"], TAB0, DUMP)

        n_c = ncnt[c]
        xT = np.zeros((ND, N_SH), np.float32)
        xT[:, :n_c] = np.asarray(x)[gb[c]:gb[c + 1]].T

        # a2a send-side: rows this core must send to each dest d = the local
        # node ids of x-rows dest d needs from us
        sg = np.zeros(C * B, np.int16)  # filled below (needs other cores)

        # pooling structures
        bl = batch[gb[c]:gb[c + 1]] - c * GS
        blp = np.full(N_SH, -1, np.int64)
        blp[:n_c] = bl
        NT = N_SH // P
        tiles = blp.reshape(NT, P)
        g_first = np.array([t[t >= 0].min() if (t >= 0).any() else 0
                            for t in tiles])
        relg = np.where(blp >= 0, blp - np.repeat(g_first, P), 255.0)
        cnt = np.bincount(bl, minlength=GS).astype(np.float32)
        gstart = np.searchsorted(bl, np.arange(GS))
        gend = np.searchsorted(bl, np.arange(GS), side="right")
        t_lo, t_hi = gstart // P, np.maximum(gend - 1, gstart) // P

        per_core.append(dict(
            gidx1=g1[0], didx1=g1[1], eaT1=g1[2], cnts1=g1[3],
            gidx2=g2[0], didx2=g2[1], eaT2=g2[2], cnts2=g2[3],
            xT=xT, uniq=pc["uniq"], uniq_owner=pc["uniq_owner"], n_c=n_c,
            relg=relg.astype(np.float32), g_first=g_first, cnt=cnt,
            t_lo=t_lo, t_hi=t_hi, sg=sg,
        ))

    # send-side gather indices: core o sends to dest c the rows c needs from o
    for o in range(C):
        sg = np.zeros(C * B, np.int16)
        for c in range(C):
            m = per_core[c]["uniq_owner"] == o
            rows = per_core[c]["uniq"][m] - gb[o]   # local node idx on o
            sg[c * B:c * B + len(rows)] = rows.astype(np.int16)
        per_core[o]["sg"] = sg

    cfg.GW = int(max((pc["relg"][pc["relg"] != 255.0]).max() + 1
                     if (pc["relg"] != 255.0).any() else 1 for pc in per_core))
    cfg.PG = int(max((pc["t_hi"] - pc["t_lo"] + 1)[pc["cnt"] > 0].max()
                     if (pc["cnt"] > 0).any() else 1 for pc in per_core))
    cfg.n_pool_idx = int(math.ceil(cfg.PG * cfg.GSP / 128) * 128)

    NT = cfg.NT
    ZPAD = NT * cfg.GW
    for pc in per_core:
        pidx = np.full(cfg.n_pool_idx, ZPAD, np.int16)
        for g in range(GS):
            if pc["cnt"][g] <= 0:
                continue
            for p, t in enumerate(range(pc["t_lo"][g], pc["t_hi"][g] + 1)):
                rel = g - pc["g_first"][t]
                pidx[p * cfg.GSP + g] = t * cfg.GW + rel
        pc["pool_idx"] = pidx
        pc["cnt_gm"] = np.maximum(
            np.pad(pc["cnt"], (0, cfg.GSP - GS)), 1.0
        ).reshape(cfg.GSP // P, P).T.astype(np.float32)

    relids = np.tile(np.arange(cfg.GW, dtype=np.float32), (P, 1))
    return cfg, gb, per_core, relids


def _gather_tables(cfg, per_core, x):
    out = []
    for pc in per_core:
        uniq = pc["uniq"]
        t0 = np.zeros((cfg.TAB0, ND), np.float32)
        t1 = np.zeros((cfg.TAB1, ND), np.float32)
        n0 = min(len(uniq), cfg.TAB0)
        t0[:n0] = x[uniq[:n0]]
        if len(uniq) > cfg.TAB0:
            t1[:len(uniq) - cfg.TAB0] = x[uniq[cfg.TAB0:]]
        out.append((t0, t1))
    return out


def _edge_phase(ctx, tc, nc, st, tabs, gidx_d, didx_d, eaT_d, w_e, acc_aps, tag,
                cnts_d=None):
    gp = ctx.enter_context(tc.tile_pool(name=f"eg{tag}", bufs=2))
    xp = ctx.enter_context(tc.tile_pool(name=f"ex{tag}", bufs=2))
    ep = ctx.enter_context(tc.tile_pool(name=f"ee{tag}", bufs=2))
    dp = ctx.enter_context(tc.tile_pool(name=f"ed{tag}", bufs=2))
    mp = ctx.enter_context(tc.tile_pool(name=f"em{tag}", bufs=2))
    pp = ctx.enter_context(tc.tile_pool(name=f"ep{tag}", bufs=2, space="PSUM"))
    for ci, (off, CH, tb) in enumerate(st.chunks):
        KB = CH // P
        gix = gp.tile([P, CH // 16], I16, tag="gix")
        nc.sync.dma_start(gix[:], gidx_d[:, off // 16:(off + CH) // 16])
        xg = xp.tile([P, KB * ND], F32, tag="xg")
        nc.gpsimd.dma_gather(
            out_ap=xg[:].rearrange("p (k e) -> p k e", e=ND),
            in_ap=tabs[tb], idxs_ap=gix[:],
            num_idxs=CH, num_idxs_reg=CH, elem_size=ND, single_packet=False)
        eat = ep.tile([ED + 1, CH], F32, tag="eat")
        nc.sync.dma_start(eat[:], eaT_d[:, off:off + CH])
        dix = dp.tile([P, CH // 16], I16, tag="dix")
        nc.sync.dma_start(dix[:], didx_d[:, off // 16:(off + CH) // 16])
        msg = mp.tile([P, KB * ND], F32, tag="msg")
        for g8 in range(0, KB, 8):
            nb = min(8, KB - g8)
            ps = pp.tile([P, 512], F32, tag="lin")
            for j in range(nb):
                b = g8 + j
                nc.tensor.matmul(out=ps[:, j * ND:(j + 1) * ND],
                                 lhsT=eat[:, b * P:(b + 1) * P],
                                 rhs=w_e[:], start=True, stop=True)
            sl = slice(g8 * ND, (g8 + nb) * ND)
            nc.vector.tensor_add(out=msg[:, sl], in0=xg[:, sl],
                                 in1=ps[:, :nb * ND])
            nc.scalar.activation(out=msg[:, sl], in_=msg[:, sl], func=AF.Relu)
        nc.gpsimd.dma_scatter_add(
            out_ap=acc_aps[ci % len(acc_aps)],
            in_ap=msg[:].rearrange("p (k e) -> p k e", e=ND),
            idxs_ap=dix[:], num_idxs=CH, num_idxs_reg=CH, elem_size=ND,
            single_packet=False)


def _zero_dram_rows(nc, t, rows, cols, zt):
    RB = 2048
    for r0 in range(0, rows, RB):
        rb = min(RB, rows - r0)
        nc.sync.dma_start(
            out=t[r0:r0 + rb, :].rearrange("(p r) e -> p (r e)", p=P),
            in_=zt[:, :rb * cols // P])


def _node_mlp(ctx, tc, nc, cfg, accs, xT_d, ident, w1, b1, w2, b2, HID,
              outT_d, last_relu, out_sbuf_cb=None, rows_cb=None):
    NCH = cfg.NCH
    ap = ctx.enter_context(tc.tile_pool(name="np_acc", bufs=3))
    xp = ctx.enter_context(tc.tile_pool(name="np_x", bufs=2))
    hp = ctx.enter_context(tc.tile_pool(name="np_h", bufs=2))
    zp = ctx.enter_context(tc.tile_pool(name="np_z", bufs=2))
    op = ctx.enter_context(tc.tile_pool(name="np_o", bufs=2))
    tp = ctx.enter_context(tc.tile_pool(name="np_tp", bufs=2, space="PSUM"))
    mp = ctx.enter_context(tc.tile_pool(name="np_mm", bufs=1, space="PSUM"))
    rp = ctx.enter_context(tc.tile_pool(name="np_r", bufs=2))

    HID2 = w2.shape[1]
    for t in range(cfg.N_SH // NCH):
        xT = xp.tile([ND, NCH], F32)
        nc.sync.dma_start(xT[:], xT_d[:, t * NCH:(t + 1) * NCH])
        at = ap.tile([P, NCH // P * ND], F32)
        nc.sync.dma_start(
            at[:].rearrange("p (j e) -> p j e", e=ND),
            accs[0][t * NCH:(t + 1) * NCH, :].rearrange("(j p) e -> p j e", p=P))
        hT = hp.tile([ND, NCH], F32)
        for j in range(NCH // P):
            pt = tp.tile([ND, P], F32, tag="tp")
            nc.tensor.transpose(out=pt[:], in_=at[:, j * ND:(j + 1) * ND],
                                identity=ident[:])
            nc.vector.tensor_add(out=hT[:, j * P:(j + 1) * P],
                                 in0=pt[:], in1=xT[:, j * P:(j + 1) * P])
        z1p = mp.tile([HID, NCH], F32, tag="mm1")
        nc.tensor.matmul(out=z1p[:], lhsT=w1[:], rhs=hT[:], start=True, stop=True)
        z1 = zp.tile([HID, NCH], F32)
        nc.scalar.activation(out=z1[:], in_=z1p[:], func=AF.Relu, bias=b1[:])
        z2p = mp.tile([HID2, NCH], F32, tag="mm2")
        nc.tensor.matmul(out=z2p[:], lhsT=w2[:], rhs=z1[:], start=True, stop=True)
        o = op.tile([HID2, NCH], F32)
        nc.scalar.activation(out=o[:], in_=z2p[:],
                             func=AF.Relu if last_relu else AF.Identity,
                             bias=b2[:])
        if outT_d is not None:
            nc.sync.dma_start(out=outT_d[:, t * NCH:(t + 1) * NCH], in_=o[:])
        if rows_cb is not None:
            # also produce node-major rows (transpose o back)
            rt = rp.tile([P, NCH // P * HID2], F32)
            for j in range(NCH // P):
                pt2 = tp.tile([P, HID2], F32, tag="tp2")
                nc.tensor.transpose(out=pt2[:], in_=o[:, j * P:(j + 1) * P],
                                    identity=ident[:HID2, :HID2])
                nc.vector.tensor_copy(out=rt[:, j * HID2:(j + 1) * HID2],
                                      in_=pt2[:])
            rows_cb(t, rt)
        if out_sbuf_cb is not None:
            out_sbuf_cb(t, o)


def _build(cfg):
    nc = bacc.Bacc("TRN2", target_bir_lowering=False, debug=False,
                   num_devices=C)
    d = {}

    def inp(name, shape, dt=F32):
        d[name] = nc.dram_tensor(name, shape, dt, kind="ExternalInput").ap()

    inp("tab0", [cfg.TAB0, ND]); inp("tab1", [cfg.TAB1, ND])
    inp("gidx1", [P, cfg.s1.ESH // 16], I16); inp("didx1", [P, cfg.s1.ESH // 16], I16)
    inp("eaT1", [ED + 1, cfg.s1.ESH])
    inp("cnts1", [1, len(cfg.s1.chunks)], mybir.dt.int32)
    inp("cnts2", [1, len(cfg.s2.chunks)], mybir.dt.int32)
    inp("gidx2", [P, cfg.s2.ESH // 16], I16); inp("didx2", [P, cfg.s2.ESH // 16], I16)
    inp("eaT2", [ED + 1, cfg.s2.ESH])
    inp("xT", [ND, cfg.N_SH])
    inp("sg", [P, C * cfg.B // 16], I16)
    inp("w_e1", [ED + 1, ND]); inp("w11", [ND, ND]); inp("b11", [ND, 1])
    inp("w12", [ND, ND]); inp("b12", [ND, 1])
    inp("w_e2", [ED + 1, ND]); inp("w21", [ND, EMB]); inp("b21", [EMB, 1])
    inp("w22", [EMB, EMB]); inp("b22", [EMB, 1])
    inp("relg", [P, cfg.NT]); inp("relids", [P, cfg.GW])
    inp("pool_idx", [P, cfg.n_pool_idx // 16], I16)
    inp("cnt_gm", [P, cfg.GSP // P]); inp("usrT", [USR, cfg.GSP])
    for nm, shp in (("hw1a", [EMB, 128]), ("hw1b", [USR, 128]), ("hb1", [128, 1]),
                    ("hw2", [128, 64]), ("hb2", [64, 1]), ("hw3", [64, 32]),
                    ("hb3", [32, 1]), ("hw4", [32, 16]), ("hb4", [16, 1]),
                    ("hw5", [16, 1]), ("hb5", [1, 1])):
        inp(nm, shp)
    yT = nc.dram_tensor("yT", [1, cfg.GSP], F32, kind="ExternalOutput").ap()

    GW, PG, NT, GSP, B = cfg.GW, cfg.PG, cfg.NT, cfg.GSP, cfg.B
    NROW = NT * GW + P

    with tile.TileContext(nc) as tc, ExitStack() as ctx:
        const = ctx.enter_context(tc.tile_pool(name="const", bufs=1))

        def ld(name, shape):
            t = const.tile(shape, F32, name=f"c_{name}")
            nc.sync.dma_start(t[:], d[name])
            return t

        w_e1 = ld("w_e1", [ED + 1, ND])
        w11 = ld("w11", [ND, ND]); b11 = ld("b11", [ND, 1])
        w12 = ld("w12", [ND, ND]); b12 = ld("b12", [ND, 1])
        w_e2 = ld("w_e2", [ED + 1, ND])
        w21 = ld("w21", [ND, EMB]); b21 = ld("b21", [EMB, 1])
        w22 = ld("w22", [EMB, EMB]); b22 = ld("b22", [EMB, 1])
        relg = ld("relg", [P, cfg.NT])
        relids = ld("relids", [P, GW])
        ident = const.tile([P, P], F32, name="ident")
        make_identity(nc, ident[:])
        zt = const.tile([P, 1024], F32, name="zt")
        nc.vector.memset(zt[:], 0.0)

        dram = ctx.enter_context(tc.tile_pool(name="dram", bufs=1, space="DRAM"))
        acc1 = dram.tile([cfg.N_SH + P, ND], F32)
        acc2 = dram.tile([cfg.N_SH + P, ND], F32)
        h1T = dram.tile([ND, cfg.N_SH], F32)
        h1r = dram.tile([cfg.N_SH, ND], F32)
        a2a_in = dram.tile([C * B, ND], F32)
        a2a_out = dram.tile([C * B, ND], F32)
        parts = dram.tile([NROW, P], F32)
        _zero_dram_rows(nc, acc1, cfg.N_SH + P, ND, zt)
        _zero_dram_rows(nc, acc2, cfg.N_SH + P, ND, zt)
        nc.sync.dma_start(
            out=parts[NT * GW:NT * GW + P, :].rearrange("(p r) e -> p (r e)", p=P),
            in_=zt[:, :P])

        # ---- layer 1 edges ----
        with ExitStack() as ectx:
            _edge_phase(ectx, tc, nc, cfg.s1, (d["tab0"], d["tab1"]),
                        d["gidx1"], d["didx1"], d["eaT1"], w_e1,
                        (acc1[:],), "1", cnts_d=d["cnts1"])

        # ---- layer 1 nodes (h1T + h1 rows) ----
        def rows_cb(t, rt):
            nc.sync.dma_start(
                out=h1r[t * cfg.NCH:(t + 1) * cfg.NCH, :]
                .rearrange("(j p) e -> p j e", p=P),
                in_=rt[:].rearrange("p (j e) -> p j e", e=ND))

        with ExitStack() as nctx:
            _node_mlp(nctx, tc, nc, cfg, (acc1,), d["xT"], ident,
                      w11, b11, w12, b12, ND, h1T[:], last_relu=True,
                      rows_cb=rows_cb)

        # ---- exchange: pre-gather + AllToAll ----
        with ExitStack() as actx:
            agp = actx.enter_context(tc.tile_pool(name="a2a", bufs=2))
            sgp = actx.enter_context(tc.tile_pool(name="a2as", bufs=2))
            for dest in range(C):
                six = sgp.tile([P, B // 16], I16, tag="six")
                nc.sync.dma_start(six[:], d["sg"][:, dest * B // 16:(dest + 1) * B // 16])
                gt = agp.tile([P, B // P * ND], F32, tag="gt")
                nc.gpsimd.dma_gather(
                    out_ap=gt[:].rearrange("p (k e) -> p k e", e=ND),
                    in_ap=h1r[:], idxs_ap=six[:],
                    num_idxs=B, num_idxs_reg=B, elem_size=ND,
                    single_packet=False)
                nc.sync.dma_start(
                    out=a2a_in[dest * B:(dest + 1) * B, :]
                    .rearrange("(k p) e -> p k e", p=P),
                    in_=gt[:].rearrange("p (k e) -> p k e", e=ND))
            nc.gpsimd.collective_compute(
                "AllToAll", mybir.AluOpType.bypass,
                replica_groups=[list(range(C))],
                ins=[a2a_in[:].opt()], outs=[a2a_out[:].opt()])

        # ---- layer 2 edges (tables = a2a_out split at TAB0) ----
        t0hi = min(cfg.TAB0, C * B)
        t2_0 = a2a_out[:t0hi, :]
        t2_1 = a2a_out[t0hi:, :] if C * B > cfg.TAB0 else t2_0
        with ExitStack() as ectx:
            _edge_phase(ectx, tc, nc, cfg.s2, (t2_0, t2_1),
                        d["gidx2"], d["didx2"], d["eaT2"], w_e2,
                        (acc2[:],), "2", cnts_d=d["cnts2"])

        # ---- layer 2 nodes + pooling partials ----
        with ExitStack() as nctx:
            pool_sb = nctx.enter_context(tc.tile_pool(name="pl_sb", bufs=2))
            pool_ps = nctx.enter_context(tc.tile_pool(name="pl_ps", bufs=1, space="PSUM"))
            pool_s = nctx.enter_context(tc.tile_pool(name="pl_s", bufs=2))
            pool_nm = nctx.enter_context(tc.tile_pool(name="pl_nm", bufs=2))

            def pool_cb(t, embT):
                npart = cfg.NCH // P
                S4 = pool_s.tile([P, npart * GW], F32, tag="S4")
                nc.vector.tensor_tensor(
                    out=S4[:].rearrange("p (j g) -> p j g", g=GW),
                    in0=relg[:, t * npart:(t + 1) * npart]
                    .unsqueeze(2).broadcast_to([P, npart, GW]),
                    in1=relids[:].unsqueeze(1).broadcast_to([P, npart, GW]),
                    op=ALU.is_equal)
                for j in range(npart):
                    tl = t * npart + j
                    tps = pool_ps.tile([P, P], F32, tag="tpose")
                    nc.tensor.transpose(out=tps[:], in_=embT[:, j * P:(j + 1) * P],
                                        identity=ident[:])
                    enm = pool_nm.tile([P, P], F32)
                    nc.vector.tensor_copy(out=enm[:], in_=tps[:])
                    pps = pool_ps.tile([GW, P], F32, tag="part")
                    nc.tensor.matmul(out=pps[:], lhsT=S4[:, j * GW:(j + 1) * GW],
                                     rhs=enm[:], start=True, stop=True)
                    psb = pool_sb.tile([GW, P], F32)
                    nc.vector.tensor_copy(out=psb[:], in_=pps[:])
                    nc.sync.dma_start(out=parts[tl * GW:(tl + 1) * GW, :],
                                      in_=psb[:])

            _node_mlp(nctx, tc, nc, cfg, (acc2,), h1T[:], ident,
                      w21, b21, w22, b22, EMB, None, last_relu=False,
                      out_sbuf_cb=pool_cb)

        # ---- pool reduce + head ----
        with ExitStack() as hctx:
            hp = hctx.enter_context(tc.tile_pool(name="hd", bufs=1))
            hps = hctx.enter_context(tc.tile_pool(name="hd_ps", bufs=2, space="PSUM"))
            pix = hp.tile([P, cfg.n_pool_idx // 16], I16)
            nc.sync.dma_start(pix[:], d["pool_idx"])
            NPB = cfg.n_pool_idx // P
            gpo = hp.tile([P, NPB * P], F32)
            nc.gpsimd.dma_gather(
                out_ap=gpo[:].rearrange("p (k e) -> p k e", e=P),
                in_ap=parts[:], idxs_ap=pix[:],
                num_idxs=cfg.n_pool_idx, num_idxs_reg=cfg.n_pool_idx,
                elem_size=P, single_packet=False)
            GB = GSP // P
            v = gpo[:].rearrange("p (q b e) -> p q b e", q=PG, b=GB)
            pooled = hp.tile([P, GB * P], F32)
            pv = pooled[:].rearrange("p (b e) -> p b e", b=GB)
            if PG == 1:
                nc.vector.tensor_copy(out=pv, in_=v[:, 0])
            else:
                nc.vector.tensor_add(out=pv, in0=v[:, 0], in1=v[:, 1])
                for q in range(2, PG):
                    nc.vector.tensor_add(out=pv, in0=pv, in1=v[:, q])
            cntg = hp.tile([P, GB], F32)
            nc.sync.dma_start(cntg[:], d["cnt_gm"])
            invc = hp.tile([P, GB], F32)
            nc.vector.reciprocal(invc[:], cntg[:])
            for b in range(GB):
                nc.vector.tensor_tensor(
                    out=pooled[:, b * P:(b + 1) * P],
                    in0=pooled[:, b * P:(b + 1) * P],
                    in1=invc[:, b:b + 1].to_broadcast([P, P]), op=ALU.mult)
            embT = hp.tile([P, GSP], F32)
            for b in range(GB):
                tps = hps.tile([P, P], F32, tag="hd")
                nc.tensor.transpose(out=tps[:], in_=pooled[:, b * P:(b + 1) * P],
                                    identity=ident[:])
                nc.vector.tensor_copy(out=embT[:, b * P:(b + 1) * P], in_=tps[:])
            usrT = hp.tile([USR, GSP], F32)
            nc.sync.dma_start(usrT[:], d["usrT"])
            hw = {nm: hp.tile(d[nm].shape, F32, name=f"t_{nm}")
                  for nm in ("hw1a", "hw1b", "hb1", "hw2", "hb2", "hw3", "hb3",
                             "hw4", "hb4", "hw5", "hb5")}
            for nm, t in hw.items():
                nc.sync.dma_start(t[:], d[nm])
            z1p = hps.tile([128, GSP], F32, tag="hd")
            nc.tensor.matmul(out=z1p[:], lhsT=hw["hw1a"][:], rhs=embT[:],
                             start=True, stop=False)
            nc.tensor.matmul(out=z1p[:], lhsT=hw["hw1b"][:], rhs=usrT[:],
                             start=False, stop=True)
            z1 = hp.tile([128, GSP], F32)
            nc.scalar.activation(out=z1[:], in_=z1p[:], func=AF.Relu, bias=hw["hb1"][:])
            z2p = hps.tile([64, GSP], F32, tag="hd")
            nc.tensor.matmul(out=z2p[:], lhsT=hw["hw2"][:], rhs=z1[:], start=True, stop=True)
            z2 = hp.tile([64, GSP], F32)
            nc.scalar.activation(out=z2[:], in_=z2p[:], func=AF.Relu, bias=hw["hb2"][:])
            z3p = hps.tile([32, GSP], F32, tag="hd")
            nc.tensor.matmul(out=z3p[:], lhsT=hw["hw3"][:], rhs=z2[:], start=True, stop=True)
            z3 = hp.tile([32, GSP], F32)
            nc.scalar.activation(out=z3[:], in_=z3p[:], func=AF.Relu, bias=hw["hb3"][:])
            z4p = hps.tile([16, GSP], F32, tag="hd")
            nc.tensor.matmul(out=z4p[:], lhsT=hw["hw4"][:], rhs=z3[:], start=True, stop=True)
            z4 = hp.tile([16, GSP], F32)
            nc.scalar.activation(out=z4[:], in_=z4p[:], func=AF.Relu, bias=hw["hb4"][:])
            z5p = hps.tile([1, GSP], F32, tag="hd")
            nc.tensor.matmul(out=z5p[:], lhsT=hw["hw5"][:], rhs=z4[:], start=True, stop=True)
            z5 = hp.tile([1, GSP], F32)
            nc.scalar.activation(out=z5[:], in_=z5p[:], func=AF.Identity, bias=hw["hb5"][:])
            nc.sync.dma_start(out=yT, in_=z5[:])

    nc.compile()
    return nc


def _make_in_maps(cfg, per_core, inputs, relids):
    f32 = lambda a: np.ascontiguousarray(np.asarray(a, np.float32))
    x = f32(inputs["x"])
    usr = f32(inputs["usr"])
    tabs = _gather_tables(cfg, per_core, x)
    w_e1 = np.vstack([f32(inputs["e1_w"]), f32(inputs["e1_b"])[None, :]])
    w_e2 = np.vstack([f32(inputs["e2_w"]), f32(inputs["e2_b"])[None, :]])
    NT = cfg.NT
    in_maps = []
    for c, pc in enumerate(per_core):
        usrT = np.zeros((USR, cfg.GSP), np.float32)
        usrT[:, :cfg.GS] = usr[c * cfg.GS:(c + 1) * cfg.GS].T
        in_maps.append(dict(
            tab0=tabs[c][0], tab1=tabs[c][1],
            gidx1=_wrap16(pc["gidx1"]), didx1=_wrap16(pc["didx1"]),
            eaT1=np.ascontiguousarray(pc["eaT1"]),
            cnts1=pc["cnts1"][None, :], cnts2=pc["cnts2"][None, :],
            gidx2=_wrap16(pc["gidx2"]), didx2=_wrap16(pc["didx2"]),
            eaT2=np.ascontiguousarray(pc["eaT2"]),
            xT=pc["xT"], sg=_wrap16(pc["sg"]),
            w_e1=w_e1, w11=f32(inputs["n1_w1"]), b11=f32(inputs["n1_b1"])[:, None],
            w12=f32(inputs["n1_w2"]), b12=f32(inputs["n1_b2"])[:, None],
            w_e2=w_e2, w21=f32(inputs["n2_w1"]), b21=f32(inputs["n2_b1"])[:, None],
            w22=f32(inputs["n2_w2"]), b22=f32(inputs["n2_b2"])[:, None],
            relg=np.ascontiguousarray(pc["relg"].reshape(NT, P).T),
            relids=relids, pool_idx=_wrap16(pc["pool_idx"]),
            cnt_gm=pc["cnt_gm"], usrT=usrT,
            hw1a=f32(inputs["h1_w"])[:EMB], hw1b=f32(inputs["h1_w"])[EMB:],
            hb1=f32(inputs["h1_b"])[:, None],
            hw2=f32(inputs["h2_w"]), hb2=f32(inputs["h2_b"])[:, None],
            hw3=f32(inputs["h3_w"]), hb3=f32(inputs["h3_b"])[:, None],
            hw4=f32(inputs["h4_w"]), hb4=f32(inputs["h4_b"])[:, None],
            hw5=f32(inputs["h5_w"]), hb5=f32(inputs["h5_b"])[:, None]))
    return in_maps


def kernel(**inputs):
    cfg, gb, per_core, relids = _preprocess(
        np.asarray(inputs["x"], np.float32), inputs["edge_index"],
        np.asarray(inputs["edge_attr"], np.float32), inputs["batch"])
    nc = _build(cfg)
    in_maps = _make_in_maps(cfg, per_core, inputs, relids)
    res = bass_utils.run_bass_kernel_spmd(nc, in_maps, core_ids=list(range(C)))
    out = np.concatenate([res.results[c]["yT"][0, :cfg.GS] for c in range(C)])
    kernel._last = res
    return out[:, None].astype(np.float32)
